# revision 7
# baseline (speedup 1.0000x reference)
"""DCGRU classifier kernel for Trainium2 (8 NeuronCores, batch-data-parallel).

Layout strategy (per core, B_loc=4 batch items), all matmuls in FP16
(fp32 matmuls run at 1/4 rate on TRN2 -- 4 cycles/row vs 1; fp16 keeps
8x the mantissa of bf16 which the 256-step recurrence needs):
  - Activations FEATURE-major: tiles are (features, batch*node) so the
    recurrent state, gates and candidate need no transposes.
  - gconv reordered as  z@(W0-W2) + S@(z@W1) + (2S^2)@(z@W2); S^T and (2S^2)^T
    are host-precomputed so the two diffusion terms are independent.
  - Projections:  q = z@W  via matmul(lhsT=zT_b, rhs=W)  -> node-major psum.
    Diffusions:   [A@q]^T via matmul(lhsT=q_b, rhs=A^T)  -> feature-major psum.
  - Software-pipelined emission: per iteration t the phases are
       P1: layer0 gate(t)   P2: layer1 cand(t-1)
       P3: layer0 cand(t)   P4: layer1 gate(t)
    so the two layers' serial chains interleave in every engine's
    in-order queue instead of running back to back.
  - GRU update h' = c + u*(h-c) on VectorE fp16, split in column halves
    so next-step projections unblock per batch-pair.
  - h2 streamed to DRAM every step; host gathers t=len-1, then
    relu->fc->maxpool tail on host (tiny).
"""

import sys

import numpy as np

sys.path.insert(0, "/opt/trn_rl_repo")

import concourse.bass as bass
import concourse.bacc as bacc
import concourse.mybir as mybir
from concourse.bass_utils import run_bass_kernel_spmd
from concourse.tile import TileContext

B, T, N, DIN, U, C = 32, 256, 128, 16, 64, 4
NCORES = 8
BL = B // NCORES  # 4 batch items per core
BN = BL * N  # 512
HB = BN // 2  # column half
F32 = mybir.dt.float32
FP16 = mybir.dt.float16
BP = np.dtype(np.float16)


# packed fp16 constant blob: (row_count, col_offset, col_count)
def _blob_layout():
    lay = {}
    col = 0

    def seg(key, rows, cols):
        nonlocal col
        lay[key] = (rows, col, cols)
        col += cols

    seg("S_T", N, N)
    seg("S2_T", N, N)
    for l, D in ((0, DIN + U), (1, 2 * U)):
        seg((l, "g12"), D, 4 * U)  # [W1 | W2] gate
        seg((l, "g0"), D, 2 * U)  # W0 - W2 gate
        seg((l, "c12"), D, 2 * U)  # [W1 | W2] cand
        seg((l, "c0"), D, U)  # W0 - W2 cand
    return lay, col


_BLOB_LAYOUT, BLOB_COLS = _blob_layout()

_NC_CACHE = {}


def _build_nc(t_steps: int):
    nc = bacc.Bacc("TRN2")

    # ---- DRAM parameters (per core) ----
    xT_e = nc.declare_dram_parameter("xT", [t_steps, DIN, BN], FP16, isOutput=False)
    blob_e = nc.declare_dram_parameter("blob", [N, BLOB_COLS], FP16, isOutput=False)
    bias_e = nc.declare_dram_parameter("biasb", [2 * U, 4], F32, isOutput=False)
    h2a_e = nc.declare_dram_parameter("h2a", [t_steps, U, BN], FP16, isOutput=True)

    with TileContext(nc) as tc:
        with (
            tc.tile_pool(name="singles", bufs=1) as singles,
            tc.tile_pool(name="sq", bufs=2) as sq_pool,
            tc.tile_pool(name="sqc", bufs=2) as sqc_pool,
            tc.tile_pool(name="sval", bufs=2) as sval_pool,
            tc.tile_pool(name="saux", bufs=4) as saux_pool,
            tc.tile_pool(name="pq", bufs=1, space="PSUM") as pq_pool,
            tc.tile_pool(name="pval", bufs=2, space="PSUM") as pval_pool,
            tc.tile_pool(name="pqc", bufs=2, space="PSUM") as pqc_pool,
            tc.tile_pool(name="pc", bufs=2, space="PSUM") as pc_pool,
        ):
            # ---- persistent tiles ----
            blob = singles.tile([N, BLOB_COLS], FP16)
            nc.sync.dma_start(out=blob, in_=blob_e[:, :])
            biasb = singles.tile([2 * U, 4], F32)
            nc.sync.dma_start(out=biasb, in_=bias_e[:, :])

            def wv(key):
                rows, c0, cols = _BLOB_LAYOUT[key]
                return blob[0:rows, c0 : c0 + cols]

            st = wv("S_T")
            s2t = wv("S2_T")
            w = {k: wv(k) for k in _BLOB_LAYOUT if isinstance(k, tuple)}

            # state tiles. Layer0 z = [h1(0:64); x(64:80)] (weight rows are
            # permuted on the host to match). Layer1 z = [h1(0:64); h2(64:128)].
            zT0 = singles.tile([DIN + U, BN], FP16)
            zcT0 = singles.tile([DIN + U, BN], FP16)
            zT1 = singles.tile([2 * U, BN], FP16)
            zcT1 = singles.tile([2 * U, BN], FP16)
            h2t = singles.tile([U, BN], FP16)  # base-0 primary copy of h2 state
            nc.vector.memset(zT0[0:U, :], 0.0)
            nc.vector.memset(zcT0[0:U, :], 0.0)
            nc.vector.memset(zT1[:, :], 0.0)
            nc.vector.memset(zcT1[:, :], 0.0)
            nc.vector.memset(h2t[:, :], 0.0)

            def gate(l, zt, zct, h, rho):
                """Projections + diffusion + sigmoid + r*h for layer l."""
                pq = pq_pool.tile([N, BL, 4 * U], F32, tag="pq")
                for b in range(BL):
                    nc.tensor.matmul(
                        pq[:, b, :],
                        lhsT=zt[:, b * N : (b + 1) * N],
                        rhs=w[l, "g12"],
                        start=True,
                        stop=True,
                    )
                pv = pval_pool.tile([2 * U, BN], F32, tag="pval")
                nc.tensor.matmul(pv, lhsT=w[l, "g0"], rhs=zt, start=True, stop=False)
                q12 = sq_pool.tile([N, BL, 4 * U], FP16, tag="q12")
                nc.scalar.copy(q12[:, 0:2, :], pq[:, 0:2, :])
                nc.vector.tensor_copy(q12[:, 2:4, :], pq[:, 2:4, :])
                for b in range(BL):
                    blk = pv[:, b * N : (b + 1) * N]
                    nc.tensor.matmul(
                        blk, lhsT=q12[:, b, 0 : 2 * U], rhs=st,
                        start=False, stop=False, skip_group_check=True,
                    )
                    nc.tensor.matmul(
                        blk, lhsT=q12[:, b, 2 * U : 4 * U], rhs=s2t,
                        start=False, stop=(b == BL - 1), skip_group_check=True,
                    )
                vs = sval_pool.tile([2 * U, BN], FP16, tag="vs")
                nc.scalar.activation(
                    vs, pv, mybir.ActivationFunctionType.Sigmoid,
                    bias=biasb[:, 2 * l : 2 * l + 1],
                )
                u0 = saux_pool.tile([U, BN], FP16, tag="u0")
                nc.vector.tensor_copy(u0, vs[U : 2 * U, :])
                # r*h into the candidate z tile
                nc.vector.tensor_mul(zct[rho : rho + U, :], vs[0:U, :], h)
                return u0

            def cand(l, zct, h, u0, out_half):
                """Candidate projections + diffusion + tanh + GRU update.

                out_half(j, val_ap) receives the updated h' column half j.
                """
                pqc = pqc_pool.tile([N, BL, 2 * U], F32, tag="pqc")
                for b in range(BL):
                    nc.tensor.matmul(
                        pqc[:, b, :],
                        lhsT=zct[:, b * N : (b + 1) * N],
                        rhs=w[l, "c12"],
                        start=True,
                        stop=True,
                    )
                pc = pc_pool.tile([U, BN], F32, tag="pc")
                nc.tensor.matmul(pc, lhsT=w[l, "c0"], rhs=zct, start=True, stop=False)
                qc = sqc_pool.tile([N, BL, 2 * U], FP16, tag="qc")
                nc.scalar.copy(qc[:, 0:2, :], pqc[:, 0:2, :])
                nc.vector.tensor_copy(qc[:, 2:4, :], pqc[:, 2:4, :])
                for b in range(BL):
                    blk = pc[:, b * N : (b + 1) * N]
                    nc.tensor.matmul(
                        blk, lhsT=qc[:, b, 0:U], rhs=st,
                        start=False, stop=False, skip_group_check=True,
                    )
                    nc.tensor.matmul(
                        blk, lhsT=qc[:, b, U : 2 * U], rhs=s2t,
                        start=False, stop=(b == BL - 1), skip_group_check=True,
                    )
                c = sval_pool.tile([U, BN], FP16, tag="c")
                nc.scalar.activation(
                    c, pc, mybir.ActivationFunctionType.Tanh,
                    bias=biasb[0:U, 2 * l + 1 : 2 * l + 2],
                )
                # h' = c + u*(h-c), per column half so consumers unblock early
                for j in range(2):
                    cs = slice(j * HB, (j + 1) * HB)
                    s_t = saux_pool.tile([U, HB], FP16, tag="s")
                    p_t = saux_pool.tile([U, HB], FP16, tag="p")
                    nc.vector.tensor_sub(s_t, h[:, cs], c[:, cs])
                    nc.vector.tensor_mul(p_t, u0[:, cs], s_t)
                    nc.vector.tensor_add(h[:, cs], c[:, cs], p_t)
                    out_half(j)

            # layer parameter tuples
            h1 = zT0[0:U, :]

            def out_half_l0(j):
                cs = slice(j * HB, (j + 1) * HB)
                nc.vector.tensor_copy(zT1[0:U, cs], zT0[0:U, cs])
                nc.vector.tensor_copy(zcT1[0:U, cs], zT0[0:U, cs])

            def out_half_l1(j):
                cs = slice(j * HB, (j + 1) * HB)
                nc.vector.tensor_copy(zT1[U : 2 * U, cs], h2t[:, cs])

            # first step's x load
            nc.sync.dma_start(out=zT0[U : U + DIN, :], in_=xT_e[0])
            nc.sync.dma_start(out=zcT0[U : U + DIN, :], in_=xT_e[0])

            u0_l1 = None
            for t in range(t_steps):
                # P1: layer0 gate(t)
                u0_l0 = gate(0, zT0, zcT0, h1, 0)
                # P2: layer1 cand(t-1)
                if t > 0:
                    cand(1, zcT1, h2t, u0_l1, out_half_l1)
                    nc.sync.dma_start(out=h2a_e[t - 1], in_=h2t)
                # P3: layer0 cand(t) -> h1(t)
                cand(0, zcT0, h1, u0_l0, out_half_l0)
                # prefetch next x (ahead of h2a store in the DMA queue)
                if t + 1 < t_steps:
                    nc.sync.dma_start(out=zT0[U : U + DIN, :], in_=xT_e[t + 1])
                    nc.sync.dma_start(out=zcT0[U : U + DIN, :], in_=xT_e[t + 1])
                # P4: layer1 gate(t)
                u0_l1 = gate(1, zT1, zcT1, h2t, U)

            # drain: layer1 cand(T-1)
            cand(1, zcT1, h2t, u0_l1, out_half_l1)
            nc.sync.dma_start(out=h2a_e[t_steps - 1], in_=h2t)

    nc.compile()
    return nc


def _prep_shared(support, W0_gate, W0_cand, W1_gate, W1_cand,
                 b0_gate, b0_cand, b1_gate, b1_cand):
    f = np.float32
    S = np.asarray(support, f)
    seg = {
        "S_T": np.ascontiguousarray(S.T),
        "S2_T": np.ascontiguousarray((2.0 * (S @ S)).T),
    }
    for l, (Wg, Wc) in enumerate(((W0_gate, W0_cand), (W1_gate, W1_cand))):
        Wg = np.asarray(Wg, f)
        Wc = np.asarray(Wc, f)
        g = [Wg[m::3] for m in range(3)]
        c = [Wc[m::3] for m in range(3)]
        if l == 0:
            # device z-layout for layer0 is [h(64); x(16)]
            perm = np.concatenate([np.arange(DIN, DIN + U), np.arange(DIN)])
            g = [gm[perm] for gm in g]
            c = [cm[perm] for cm in c]
        seg[(l, "g12")] = np.concatenate([g[1], g[2]], axis=1)
        seg[(l, "g0")] = g[0] - g[2]
        seg[(l, "c12")] = np.concatenate([c[1], c[2]], axis=1)
        seg[(l, "c0")] = c[0] - c[2]
    blob = np.zeros((N, BLOB_COLS), BP)
    for key, (rows, c0, cols) in _BLOB_LAYOUT.items():
        a = seg[key]
        assert a.shape == (rows, cols), (key, a.shape, rows, cols)
        blob[:rows, c0 : c0 + cols] = a.astype(BP)
    biasb = np.zeros((2 * U, 4), f)
    biasb[:, 0] = np.asarray(b0_gate, f).reshape(-1)
    biasb[0:U, 1] = np.asarray(b0_cand, f).reshape(-1)
    biasb[:, 2] = np.asarray(b1_gate, f).reshape(-1)
    biasb[0:U, 3] = np.asarray(b1_cand, f).reshape(-1)
    return {"blob": blob, "biasb": biasb}


def run_cores(inputs, t_steps=T, trace=False):
    """Build in_maps, run the SPMD kernel, return per-core results."""
    input_seq = np.asarray(inputs["input_seq"], np.float32)
    shared = _prep_shared(
        inputs["support"], inputs["W0_gate"], inputs["W0_cand"],
        inputs["W1_gate"], inputs["W1_cand"],
        inputs["b0_gate"], inputs["b0_cand"], inputs["b1_gate"], inputs["b1_cand"],
    )
    in_maps = []
    for k in range(NCORES):
        xs = input_seq[k * BL : (k + 1) * BL, :t_steps]  # (BL, t, N, DIN)
        xT = np.ascontiguousarray(
            np.transpose(xs, (1, 3, 0, 2)).reshape(t_steps, DIN, BN)
        ).astype(BP)
        in_maps.append(dict(shared, xT=xT))
    if t_steps not in _NC_CACHE:
        _NC_CACHE[t_steps] = _build_nc(t_steps)
    nc = _NC_CACHE[t_steps]
    res = run_bass_kernel_spmd(nc, in_maps, list(range(NCORES)), trace=trace)
    return res


def finish_host(results, inputs, t_steps=T):
    """Host tail: gather t=len-1, relu -> fc -> node max-pool."""
    W_fc = np.asarray(inputs["W_fc"], np.float32)
    b_fc = np.asarray(inputs["b_fc"], np.float32)
    seq_lengths = np.asarray(inputs["seq_lengths"]).astype(np.int64)
    out = np.empty((B, C), np.float32)
    for k in range(NCORES):
        h2a = np.asarray(results[k]["h2a"])  # (T, U, BN) fp16
        for b in range(BL):
            tt = int(min(seq_lengths[k * BL + b], t_steps) - 1)
            blk = h2a[tt, :, b * N : (b + 1) * N].astype(np.float32).T  # (N, U)
            logits = np.maximum(blk, 0.0) @ W_fc + b_fc  # (N, C)
            out[k * BL + b] = logits.max(axis=0)
    return out


def kernel(**inputs):
    res = run_cores(inputs, t_steps=T)
    return finish_host(res.results, inputs, t_steps=T)
